# revision 1
# baseline (speedup 1.0000x reference)
"""CrossCLR intra-modality loss on 8 Trainium2 NeuronCores — fp8 edition.

Data-parallel over the 4096-row batch (512 rows/core). Host prep
normalizes each embedding row and quantizes to fp8-e4m3 (layout/
precision prep, like the baseline's transpose+bf16 cast), so every
on-device matmul produces cosines directly and runs in DoubleRow perf
mode (4x bf16 MACs/cycle, K=256/instruction). Each core computes its
row-slab of G = post_l @ brand^T once; the transposed-side stats the
brand loss needs come from PE ones-matmul partition reductions and a
rank-major [64,128] ReduceScatter (each core receives exactly its own
512 columns). Rank counts use exact f32 threshold algebra:
  S_ij > S_ii  <=>  cos_ij > (cos_ii*nB_i) * invB_j   (row count)
  S_ij > S_jj  <=>  cos_ij > (cos_jj*nP_j) * invP_i   (col indicator)
each a single DVE scalar_tensor_tensor on the PSUM tile, with the
column threshold row AllGathered from the per-core diag. The gram
negatives C/D exp straight out of PSUM on ACT (constant scale) and are
diag-masked post-exp by one bf16 2x-packed stt with accumulate; the
reference's exp(0)=1 diag contribution is restored as +1.0 in the
final logsumexp. Phase order C -> G -> D hides both collective
latencies behind collective-independent gram compute. Host sums the
per-core [128, 2*MT] losses and halves.
"""

import sys

sys.path.insert(0, "/opt/trn_rl_repo")

from contextlib import ExitStack
from functools import lru_cache

import ml_dtypes
import numpy as np

import concourse.bacc as bacc
import concourse.mybir as mybir
import concourse.tile as tile
from concourse.bass_utils import run_bass_kernel_spmd

N = 4096          # batch rows (global)
D = 1024          # embedding dim
NC = 8            # cores
S = N // NC       # local rows per core (512)
P = 128           # partitions
KC = D // P       # contraction chunks (8)
MT = S // P       # local row tiles (4)
SEG = 512         # PSUM bank width (f32)
SEGW = 1024       # epilogue tile width (two banks)
NSEGW = N // SEGW  # 4
TEMP = 0.03
NEG_W = 0.8
MASK_W = N + 384  # sliding diag-mask width

F32 = mybir.dt.float32
BF16 = mybir.dt.bfloat16
FP8 = mybir.dt.float8e4
AF = mybir.ActivationFunctionType
OP = mybir.AluOpType
PM = mybir.MatmulPerfMode.DoubleRow
E4M3 = ml_dtypes.float8_e4m3
BF16_NP = ml_dtypes.bfloat16


def build_program():
    nc = bacc.Bacc("TRN2", target_bir_lowering=False, debug=False, num_devices=NC)

    bTn_d = nc.dram_tensor("bTn", (D, N), FP8, kind="ExternalInput")
    pTn_d = nc.dram_tensor("pTn", (D, N), FP8, kind="ExternalInput")
    bTnl_d = nc.dram_tensor("bTnl", (D, S), FP8, kind="ExternalInput")
    pTnl_d = nc.dram_tensor("pTnl", (D, S), FP8, kind="ExternalInput")
    cmask_d = nc.dram_tensor("cmask", (P, MASK_W), BF16, kind="ExternalInput")
    invBrow_d = nc.dram_tensor("invBrow", (1, N), F32, kind="ExternalInput")
    trowin_d = nc.dram_tensor("trowin", (1, N), F32, kind="ExternalInput")
    diagP_d = nc.dram_tensor("diagP", (P, MT), F32, kind="ExternalInput")
    ci_d = nc.dram_tensor("ci", (P, MT), F32, kind="ExternalInput")
    invPl_d = nc.dram_tensor("invPl", (P, MT), F32, kind="ExternalInput")
    out_d = nc.dram_tensor("out", (P, 5 * MT), F32, kind="ExternalOutput")

    # DRAM scratch (collectives can't touch I/O tensors)
    cs_d = nc.dram_tensor("cs", (2 * MT * 2 * NSEGW, P), F32)  # rank-major col stats
    rs_d = nc.dram_tensor("rs", (2 * MT, P), F32)              # this core's share

    with tile.TileContext(nc) as tc, ExitStack() as ctx:
        pin = ctx.enter_context(tc.tile_pool(name="pin", bufs=1))
        pstat = ctx.enter_context(tc.tile_pool(name="pstat", bufs=1))
        pwork = ctx.enter_context(tc.tile_pool(name="pwork", bufs=2))
        pex = ctx.enter_context(tc.tile_pool(name="pex", bufs=4))
        pind = ctx.enter_context(tc.tile_pool(name="pind", bufs=4))
        pjunk = ctx.enter_context(tc.tile_pool(name="pjunk", bufs=4))

        # ---- resident inputs. HWDGE issue costs ~630ns serialized on the
        # issuing queue, so batch loads into few big DMAs (AP rearrange) and
        # keep the ACT queue completely free of input DMAs — it must start
        # phase C's exps as early as possible. ----
        def load_cols(eng, dst, src_d, c0, c1):
            eng.dma_start(out=dst[:, :, c0:c1],
                          in_=src_d.ap()[:, c0:c1]
                          .rearrange("(k p) n -> p k n", p=P))

        # The DMA engine is a serial FIFO resource: issue order = transfer
        # order. Choreography: slabs (head the AllGather's critical chain),
        # bTn first half (phase C), cmask, then phase-1's tiny collective
        # feeds, then the rest of the bulk. pTn rides the gpsimd queue
        # BEHIND the trow broadcast so the post-AllGather read isn't stuck
        # in the FIFO behind it.
        bTnl = pin.tile([P, KC, S], FP8, tag="bTnl")
        pTnl = pin.tile([P, KC, S], FP8, tag="pTnl")
        load_cols(nc.sync, bTnl, bTnl_d, 0, S)
        load_cols(nc.sync, pTnl, pTnl_d, 0, S)
        bTn = pin.tile([P, KC, N], FP8, tag="bTn")
        pTn = pin.tile([P, KC, N], FP8, tag="pTn")
        load_cols(nc.sync, bTn, bTn_d, 0, N // 4)
        load_cols(nc.sync, bTn, bTn_d, N // 4, N // 2)
        ones = pin.tile([P, 1], BF16, tag="ones")
        nc.vector.memset(ones[:], 1.0)

        # phase C's remaining needs first: cmask then the bTn second half
        cmask = pin.tile([P, MASK_W], BF16, tag="cmask")
        nc.sync.dma_start(out=cmask[:], in_=cmask_d[:, :])
        load_cols(nc.sync, bTn, bTn_d, N // 2, N)
        invPl = pstat.tile([P, MT], F32, tag="invPl")
        nc.sync.dma_start(out=invPl[:], in_=invPl_d[:, :])
        diagP = pstat.tile([P, MT], F32, tag="diagP")
        nc.sync.dma_start(out=diagP[:], in_=diagP_d[:, :])
        ci = pstat.tile([P, MT], F32, tag="ci")
        nc.sync.dma_start(out=ci[:], in_=ci_d[:, :])
        # trow (column thresholds) and diag are host-computed properties of
        # the quantized inputs, so no AllGather is needed; only the column-
        # stat ReduceScatter remains as a collective. Broadcast rows load in
        # interleaved halves so phase G's first tiles unblock early.
        trow = pin.tile([P, N], F32, tag="trow")
        invBrow = pin.tile([P, N], F32, tag="invBrow")
        nc.sync.dma_start(
            out=invBrow[:, 0:N // 2],
            in_=invBrow_d.ap()[0:1, 0:N // 2].partition_broadcast(P))
        nc.sync.dma_start(
            out=trow[:, 0:N // 2],
            in_=trowin_d.ap()[0:1, 0:N // 2].partition_broadcast(P))
        nc.sync.dma_start(
            out=invBrow[:, N // 2:N],
            in_=invBrow_d.ap()[0:1, N // 2:N].partition_broadcast(P))
        nc.sync.dma_start(
            out=trow[:, N // 2:N],
            in_=trowin_d.ap()[0:1, N // 2:N].partition_broadcast(P))
        # fabricated dependencies: the scheduler issues ready DMAs
        # immediately and the serial DMA engine is FIFO, so without these
        # the bulk pTn loads would jam the queue ahead of the collective
        # feeds and the trow broadcast. The 4-byte casting DMAs below are
        # overwritten by the real loads; they only chain pTn behind trow
        # (one sliver per half — the dependency is regional).
        nc.gpsimd.dma_start(out=pTn[0:1, 0, 0:4], in_=trow[0:1, 0:4])
        nc.gpsimd.dma_start(out=pTn[0:1, 0, N // 2:N // 2 + 4],
                            in_=trow[0:1, N // 2:N // 2 + 4])
        load_cols(nc.gpsimd, pTn, pTn_d, 0, N // 2)
        load_cols(nc.gpsimd, pTn, pTn_d, N // 2, N)

        # stat accumulators, reduced in one op each at the end
        SEGW2 = 1024      # C/D epilogue width
        NSEGW2 = N // SEGW2
        cntP, sG = (pstat.tile([P, MT, NSEGW], F32, tag=n, name=n)
                    for n in ("cntP", "sG"))
        sC, sD = (pstat.tile([P, MT, NSEGW2], F32, tag=n, name=n)
                  for n in ("sC", "sD"))

        def mm(ps, lhsT, rhs, mt, col0, width):
            for h in range(width // SEG):
                for kk in range(KC // 2):
                    nc.tensor.matmul(
                        ps[:, h * SEG:(h + 1) * SEG],
                        lhsT[:, 2 * kk:2 * kk + 2, mt * P:(mt + 1) * P],
                        rhs[:, 2 * kk:2 * kk + 2,
                            col0 + h * SEG:col0 + (h + 1) * SEG],
                        start=(kk == 0), stop=(kk == KC // 2 - 1), perf_mode=PM)

        def gram(lhsT, rhs, sum_t, pool):
            """negatives slab: diag-masked on PSUM (DVE first — it's the
            bottleneck engine), then in-place exp with row-sum accumulate;
            the zeroed diag contributes exp(0)=1 exactly as the reference"""
            for sw in range(NSEGW2):
                for mt in range(MT):
                    ps = pool.tile([P, SEGW2], F32, tag="big")
                    mm(ps, lhsT, rhs, mt, sw * SEGW2, SEGW2)
                    moff = 384 - P * mt
                    t = pjunk.tile([P, SEGW2], BF16, tag="junk")
                    nc.vector.scalar_tensor_tensor(
                        out=t[:], in0=ps[:], scalar=1.0,
                        in1=cmask[:, moff + sw * SEGW2:moff + (sw + 1) * SEGW2],
                        op0=OP.mult, op1=OP.mult)
                    nc.scalar.activation(out=t[:], in_=t[:], func=AF.Exp,
                                         scale=NEG_W / TEMP,
                                         accum_out=sum_t[:, mt, sw:sw + 1])

        # ---- phase C: brand gram negatives (covers the AllGather) ----
        with tc.tile_pool(name="pbigC", bufs=3, space="PSUM") as pbigC:
            gram(bTnl, bTn, sC, pbigC)

        # ---- phase G: G = post_l @ brand^T with fused row+col stats.
        # The PE queue is in-order: a tile's ones-matmuls wait on DVE/ACT
        # outputs, so emit them one tile LATE to keep DR matmuls at the
        # queue head (software pipelining). Same for the sw-boundary PSUM
        # copies on the ACT queue. ----
        from collections import deque

        # two col-stat accumulators share one PSUM bank (partitions 0 and 64;
        # matmul out base must be 0/32/64), so pcol takes 2 banks not 4 and
        # the big-psum pool gets a third slot
        with tc.tile_pool(name="pbigG", bufs=3, space="PSUM") as pbigG, \
             tc.tile_pool(name="pcol", bufs=1, space="PSUM") as pcol:
            pend_pe = deque()
            pend_act = None
            for sw in range(NSEGW):
                pcA = pcol.tile([P, SEG], F32, tag="pcA", name="pcA")
                pcB = pcol.tile([P, SEG], F32, tag="pcB", name="pcB")
                # (tile, row) per stat: cnt_lo/exp_lo in A, cnt_hi/exp_hi in B
                pcs = [(pcA, 0), (pcB, 0), (pcA, 64), (pcB, 64)]
                for mt in range(MT):
                    ps = pbigG.tile([P, SEGW], F32, tag="big")
                    mm(ps, pTnl, bTn, mt, sw * SEGW, SEGW)
                    while len(pend_pe) >= 2:
                        pend_pe.popleft()()
                    # exp straight off PSUM (inputs are pre-normalized)
                    ex = pex.tile([P, SEGW], BF16, tag="ex")
                    nc.scalar.activation(out=ex[:], in_=ps[:], func=AF.Exp,
                                         scale=1.0 / TEMP,
                                         accum_out=sG[:, mt, sw:sw + 1])
                    # flush the previous sw's copies only after its last
                    # ones-matmul (their producer) has been emitted (mt==1
                    # pops it from the lag-2 deque above)
                    if mt == 1 and pend_act is not None:
                        pend_act()
                        pend_act = None
                    # col indicator: cos_ij > trow_j * invP_i
                    ind = pind.tile([P, SEGW], BF16, tag="ind")
                    nc.vector.scalar_tensor_tensor(
                        out=ind[:], in0=trow[:, sw * SEGW:(sw + 1) * SEGW],
                        scalar=invPl[:, mt:mt + 1], in1=ps[:],
                        op0=OP.mult, op1=OP.is_lt)
                    # row count: cos_ij > ci_i * invBrow_j
                    junk = pjunk.tile([P, SEGW], BF16, tag="junk")
                    nc.vector.scalar_tensor_tensor(
                        out=junk[:], in0=invBrow[:, sw * SEGW:(sw + 1) * SEGW],
                        scalar=ci[:, mt:mt + 1], in1=ps[:],
                        op0=OP.mult, op1=OP.is_lt,
                        accum_out=cntP[:, mt, sw:sw + 1])

                    def ones_mms(pcs=pcs, ind=ind, ex=ex, mt=mt):
                        for h in range(2):
                            t_i, r_i = pcs[h]
                            nc.tensor.matmul(t_i[r_i:r_i + 1, :], ones[:],
                                             ind[:, h * SEG:(h + 1) * SEG],
                                             start=(mt == 0), stop=(mt == MT - 1))
                            t_e, r_e = pcs[2 + h]
                            nc.tensor.matmul(t_e[r_e:r_e + 1, :], ones[:],
                                             ex[:, h * SEG:(h + 1) * SEG],
                                             start=(mt == 0), stop=(mt == MT - 1))
                    pend_pe.append(ones_mms)

                def copies(pcA=pcA, pcB=pcB, sw=sw):
                    # one wide copy per bank (rows 0=cnt, 64=exp), then the
                    # rank-major scatter: seg=2*sw+h, cnt rows seg*8+[0:4),
                    # exp rows seg*8+[4:8)
                    for h, src in ((0, pcA), (1, pcB)):
                        seg = 2 * sw + h
                        b = pwork.tile([65, SEG], F32, tag="csb")
                        nc.scalar.activation(out=b[:], in_=src[0:65, :],
                                             func=AF.Copy)
                        nc.sync.dma_start(
                            out=cs_d[seg * 8:seg * 8 + 4, :], in_=b[0:1, :])
                        nc.sync.dma_start(
                            out=cs_d[seg * 8 + 4:seg * 8 + 8, :], in_=b[64:65, :])
                pend_act = copies
            while pend_pe:
                pend_pe.popleft()()
            if pend_act is not None:
                pend_act()

        # each core receives exactly its own column segment's stats
        nc.gpsimd.collective_compute(
            "ReduceScatter", OP.add, replica_groups=[list(range(NC))],
            ins=[cs_d.ap().opt()], outs=[rs_d.ap().opt()])

        # ---- phase D: post gram negatives (covers the ReduceScatter) ----
        with tc.tile_pool(name="pbigD", bufs=3, space="PSUM") as pbigD:
            gram(pTnl, pTn, sD, pbigD)

        # ---- phase 4: ship per-row stats (counts, exp sums, diag); the
        # O(N) ln/rank epilogue runs on host. RS-independent reductions are
        # emitted first so the DVE queue doesn't head-block on csl. ----
        out_sb = pstat.tile([P, 5 * MT], F32, tag="out_sb")
        nc.vector.tensor_reduce(out_sb[:, MT:2 * MT], cntP[:, :, :],
                                mybir.AxisListType.X, OP.add)
        red = {}
        for name, t in (("sG", sG), ("sC", sC), ("sD", sD)):
            r = pstat.tile([P, MT], F32, tag=f"r{name}")
            nc.vector.tensor_reduce(r[:], t[:, :, :], mybir.AxisListType.X, OP.add)
            red[name] = r
        # post side exp sum: G row sums + D row sums
        nc.vector.tensor_tensor(out_sb[:, 3 * MT:4 * MT], red["sG"][:],
                                red["sD"][:], OP.add)
        nc.vector.tensor_scalar(out=out_sb[:, 4 * MT:5 * MT], in0=diagP[:],
                                scalar1=1.0, scalar2=None, op0=OP.mult)

        csl = pstat.tile([P, 2 * MT], F32, tag="csl")
        nc.sync.dma_start(out=csl[:], in_=rs_d.ap().transpose([1, 0]))
        nc.vector.tensor_scalar(out=out_sb[:, 0:MT], in0=csl[:, 0:MT],
                                scalar1=0.0, scalar2=None, op0=OP.add)
        # brand side exp sum: gathered col expsums + C row sums
        nc.vector.tensor_tensor(out_sb[:, 2 * MT:3 * MT], csl[:, MT:2 * MT],
                                red["sC"][:], OP.add)
        nc.sync.dma_start(out=out_d[:, :], in_=out_sb[:])

    nc.compile()
    return nc


@lru_cache(maxsize=1)
def _program():
    return build_program()


def _core_inputs(brand, post):
    nB = np.linalg.norm(brand, axis=1)
    nP = np.linalg.norm(post, axis=1)
    bTn = np.ascontiguousarray((brand / nB[:, None]).T).astype(E4M3)
    pTn = np.ascontiguousarray((post / nP[:, None]).T).astype(E4M3)
    invB = (1.0 / nB).astype(np.float32).reshape(1, N)
    # threshold prep from the quantized data (exact f32, like the norms)
    diag_full = (bTn.astype(np.float32) * pTn.astype(np.float32)).sum(0)
    trow_in = (diag_full * nP).astype(np.float32).reshape(1, N)
    maps = []
    for c in range(NC):
        cmask = np.ones((P, MASK_W), dtype=BF16_NP)
        rows = np.arange(P)
        cmask[rows, 384 + S * c + rows] = 0.0
        maps.append({
            "bTn": bTn,
            "pTn": pTn,
            "bTnl": np.ascontiguousarray(bTn[:, S * c:S * (c + 1)]),
            "pTnl": np.ascontiguousarray(pTn[:, S * c:S * (c + 1)]),
            "cmask": cmask,
            "invBrow": invB,
            "trowin": trow_in,
            "diagP": np.ascontiguousarray(
                diag_full[S * c:S * (c + 1)].reshape(MT, P).T
            ).astype(np.float32),
            "ci": np.ascontiguousarray(
                (diag_full[S * c:S * (c + 1)] * nB[S * c:S * (c + 1)])
                .reshape(MT, P).T).astype(np.float32),
            "invPl": np.ascontiguousarray(
                (1.0 / nP[S * c:S * (c + 1)]).reshape(MT, P).T
            ).astype(np.float32),
        })
    return maps


def kernel(brand, post):
    brand = np.asarray(brand, dtype=np.float32)
    post = np.asarray(post, dtype=np.float32)
    nc = _program()
    res = run_bass_kernel_spmd(nc, _core_inputs(brand, post), list(range(NC)))
    total = 0.0
    for r in res.results:
        o = r["out"].astype(np.float64)
        cnt = o[:, 0:2 * MT]                      # brand col counts | post row counts
        es = o[:, 2 * MT:4 * MT]                  # brand | post exp sums
        diag = np.tile(o[:, 4 * MT:5 * MT], (1, 2))
        loss = (1.0 + 1.0 / (N - cnt)) * (np.log(es) - diag / TEMP)
        total += loss.sum()
    return np.float32(total / 2.0)



# revision 11
# speedup vs baseline: 1.5281x; 1.5281x over previous
"""CrossCLR intra-modality loss on 8 Trainium2 NeuronCores — v3.

Data-parallel over the 4096-row batch (512 rows/core), fp8-e4m3
normalized embeddings, DoubleRow matmuls. Structure:

* Column rotation: per-core inputs are host-rolled by c*512 columns, so
  each core's slab sits at columns [0, 512) and gram diagonals at the
  fixed position col = mt*128 + p. One SPMD instruction stream.
* Symmetric gram halving: the brand gram C and post gram D are
  symmetric; each core computes only rotated cols [0, 2560) (diag
  block + 4 neighbor slabs; the distance-4 block is computed by both
  partners for their own row sums). Row-sum contributions of the
  uncomputed distance 5..7 blocks come from partners' column sums over
  [512, 2048) (distances 1..3), exchanged through the host.
* Diag masking on PE: a [128, 512] identity-matmul accumulates -16
  onto the diag strip in PSUM, exp underflows to exactly 0 there, and
  the reference's exp(0)=1 diag is restored as +1.0 on the host.
* Merged software pipeline: C, G(=post_l @ brand^T), and D tiles are
  emitted interleaved (PSUM 4+4 banks), so the ACT-heavy gram exp work
  overlaps the DVE-heavy rank-threshold compares of G, keeping every
  engine fed from one end of the kernel to the other.
* No collective and no PE partition-reductions: mt-paired bf16 stat
  tiles (column-count indicators, exp sums) are DMA'd to DRAM and the
  HOST does the partition + cross-core reduction and the O(N)
  rank/log epilogue.

Engines: ACT = all exps (+row-sum accum); DVE = the two rank compares
(+row-count accum) and exp pair-adds; GPSIMD = indicator pair-adds and
gram colsum adds (SBUF-only; GPSIMD cannot touch PSUM); PE = matmuls.
"""

import sys

sys.path.insert(0, "/opt/trn_rl_repo")

from contextlib import ExitStack
from functools import lru_cache

import ml_dtypes
import numpy as np

import concourse.bacc as bacc
import concourse.mybir as mybir
import concourse.tile as tile
from concourse.bass_utils import run_bass_kernel_spmd

N = 4096          # batch rows (global)
D = 1024          # embedding dim
NC = 8            # cores
S = N // NC       # local rows per core (512)
P = 128           # partitions
KC = D // P       # contraction chunks (8)
MT = S // P       # local row tiles (4)
SEG = 512         # PSUM bank width (f32)
SEGW = 1024       # tile width (two banks)
NSEGW = N // SEGW  # 4
CW = 2560         # C/D computed column window (rotated)
CS0, CS1 = 512, 2048  # C/D colsum emission window (distances 1..3)
CWIN = CS1 - CS0  # 1536
TEMP = 0.03
NEG_W = 0.8
BNEG = -16.0      # diag bias: exp(w*(cos-16)/T) underflows to 0

F32 = mybir.dt.float32
BF16 = mybir.dt.bfloat16
FP8 = mybir.dt.float8e4
AF = mybir.ActivationFunctionType
OP = mybir.AluOpType
PM = mybir.MatmulPerfMode.DoubleRow
E4M3 = ml_dtypes.float8_e4m3
BF16_NP = ml_dtypes.bfloat16


def build_program():
    nc = bacc.Bacc("TRN2", target_bir_lowering=False, debug=False, num_devices=NC)

    bTn_d = nc.dram_tensor("bTn", (D, N), FP8, kind="ExternalInput")
    pTn_d = nc.dram_tensor("pTn", (D, CW), FP8, kind="ExternalInput")
    trow_d = nc.dram_tensor("trow", (1, N), BF16, kind="ExternalInput")
    invB_d = nc.dram_tensor("invB", (1, N), BF16, kind="ExternalInput")
    ident_d = nc.dram_tensor("ident", (P, P), BF16, kind="ExternalInput")
    dmask_d = nc.dram_tensor("dmask", (P, 896), BF16, kind="ExternalInput")
    ci_d = nc.dram_tensor("ci", (P, MT), F32, kind="ExternalInput")
    invPl_d = nc.dram_tensor("invPl", (P, MT), F32, kind="ExternalInput")
    outA_d = nc.dram_tensor("outA", (P, 4 * MT), F32, kind="ExternalOutput")
    # per-sw pair tiles: [ind01, ind23, exp01, exp23] x P rows, rotated cols
    gcol_d = nc.dram_tensor("gcol", (NSEGW * 4 * P, SEGW), BF16,
                            kind="ExternalOutput")
    # gram colsums over [CS0, CS1): rows [0:P) = C, [P:2P) = D
    csum_d = nc.dram_tensor("csum", (2 * P, CWIN), BF16, kind="ExternalOutput")

    with tile.TileContext(nc) as tc, ExitStack() as ctx:
        pin = ctx.enter_context(tc.tile_pool(name="pin", bufs=1))
        pstat = ctx.enter_context(tc.tile_pool(name="pstat", bufs=1))
        pex = ctx.enter_context(tc.tile_pool(name="pex", bufs=3))
        pexcd = ctx.enter_context(tc.tile_pool(name="pexcd", bufs=5))
        pind = ctx.enter_context(tc.tile_pool(name="pind", bufs=3))
        pacc = ctx.enter_context(tc.tile_pool(name="pacc", bufs=2))
        pjunk = ctx.enter_context(tc.tile_pool(name="pjunk", bufs=4))

        def load_cols(eng, dst, src_d, c0, c1):
            eng.dma_start(out=dst[:, :, c0:c1],
                          in_=src_d.ap()[:, c0:c1]
                          .rearrange("(k p) n -> p k n", p=P))

        # ---- input loads: one serial DMA resource; priority order matches
        # first use in the merged pipeline ----
        bTn = pin.tile([P, KC, N], FP8, tag="bTn")
        pTn = pin.tile([P, KC, CW], FP8, tag="pTn")
        load_cols(nc.sync, bTn, bTn_d, 0, SEG)
        load_cols(nc.sync, bTn, bTn_d, SEG, SEGW)
        load_cols(nc.sync, pTn, pTn_d, 0, S)
        ident = pin.tile([P, P], BF16, tag="ident")
        nc.sync.dma_start(out=ident[:], in_=ident_d[:, :])
        dmask = pin.tile([P, 896], BF16, tag="dmask")
        nc.sync.dma_start(out=dmask[:], in_=dmask_d[:, :])
        ci = pstat.tile([P, MT], F32, tag="ci")
        nc.sync.dma_start(out=ci[:], in_=ci_d[:, :])
        invPl = pstat.tile([P, MT], F32, tag="invPl")
        nc.sync.dma_start(out=invPl[:], in_=invPl_d[:, :])
        trow = pin.tile([P, N], BF16, tag="trow")
        invB = pin.tile([P, N], BF16, tag="invB")
        nc.sync.dma_start(out=trow[:, 0:SEGW],
                          in_=trow_d.ap()[0:1, 0:SEGW].partition_broadcast(P))
        nc.sync.dma_start(out=invB[:, 0:SEGW],
                          in_=invB_d.ap()[0:1, 0:SEGW].partition_broadcast(P))
        load_cols(nc.sync, bTn, bTn_d, SEGW, 2 * SEGW)
        load_cols(nc.sync, bTn, bTn_d, 2 * SEGW, CW)
        nc.sync.dma_start(out=trow[:, SEGW:2 * SEGW],
                          in_=trow_d.ap()[0:1, SEGW:2 * SEGW].partition_broadcast(P))
        nc.sync.dma_start(out=invB[:, SEGW:2 * SEGW],
                          in_=invB_d.ap()[0:1, SEGW:2 * SEGW].partition_broadcast(P))
        load_cols(nc.sync, bTn, bTn_d, CW, 3 * SEGW)
        load_cols(nc.sync, bTn, bTn_d, 3 * SEGW, N)
        nc.sync.dma_start(out=trow[:, 2 * SEGW:N],
                          in_=trow_d.ap()[0:1, 2 * SEGW:N].partition_broadcast(P))
        nc.sync.dma_start(out=invB[:, 2 * SEGW:N],
                          in_=invB_d.ap()[0:1, 2 * SEGW:N].partition_broadcast(P))
        load_cols(nc.sync, pTn, pTn_d, S, 3 * SEG)
        load_cols(nc.sync, pTn, pTn_d, 3 * SEG, 2 * SEGW)
        load_cols(nc.sync, pTn, pTn_d, 2 * SEGW, CW)

        ones = pin.tile([P, 1], BF16, tag="ones")
        nc.vector.memset(ones[:], 1.0)

        cntP, sG = (pstat.tile([P, MT, NSEGW], F32, tag=n, name=n)
                    for n in ("cntP", "sG"))
        sC, sD = (pstat.tile([P, MT, 3], F32, tag=n, name=n)
                  for n in ("sC", "sD"))
        csumC = pstat.tile([P, CWIN], BF16, tag="csumC")
        csumD = pstat.tile([P, CWIN], BF16, tag="csumD")

        def mm(ps, lhsT, rhs, mt, col0, width, open_segs=()):
            for h in range(width // SEG):
                for kk in range(KC // 2):
                    nc.tensor.matmul(
                        ps[:, h * SEG:(h + 1) * SEG],
                        lhsT[:, 2 * kk:2 * kk + 2, mt * P:(mt + 1) * P],
                        rhs[:, 2 * kk:2 * kk + 2,
                            col0 + h * SEG:col0 + (h + 1) * SEG],
                        start=(kk == 0),
                        stop=(kk == KC // 2 - 1 and h not in open_segs),
                        perf_mode=PM)

        # ---- C/D gram unit: (matrix, mt, chunk) with chunks of
        # [0,1024), [1024,2048), [2048,2560) rotated cols ----
        cd_state = {}

        def cd_unit(which, mt, ch, pcd):
            src = bTn if which == "C" else pTn
            slots = sC if which == "C" else sD
            csum = csumC if which == "C" else csumD
            st = cd_state.setdefault(which, {"exs": {}, "p01": {}})
            width = SEG if ch == 2 else SEGW
            ps = pcd.tile([P, SEGW], F32, tag="cd", name="cd")
            mm(ps, src, src, mt, ch * SEGW, width,
               open_segs=(0,) if ch == 0 else ())
            if ch == 0:
                dm = dmask[:, 384 - mt * P:384 - mt * P + SEG]
                nc.tensor.matmul(ps[:, 0:SEG], ident[:], dm,
                                 start=False, stop=True)
            if ch == 2:
                junk = pjunk.tile([P, SEG], BF16, tag="junkcd", name="junkcd")
                nc.scalar.activation(out=junk[:], in_=ps[:, 0:SEG],
                                     func=AF.Exp, scale=NEG_W / TEMP,
                                     accum_out=slots[:, mt, ch:ch + 1])
                return
            ex = pexcd.tile([P, SEGW], BF16, tag="excd", name="excd")
            nc.scalar.activation(out=ex[:], in_=ps[:], func=AF.Exp,
                                 scale=NEG_W / TEMP,
                                 accum_out=slots[:, mt, ch:ch + 1])
            # colsum window: ch0 contributes [512,1024), ch1 all of
            # [1024,2048). Pairwise adds on GPSIMD (SBUF-only engine).
            w = slice(SEG, SEGW) if ch == 0 else slice(0, SEGW)
            ww = SEG if ch == 0 else SEGW
            co = 0 if ch == 0 else SEG  # offset into csum tile
            if mt % 2 == 0:
                st["exs"][ch] = ex
            else:
                prev = st["exs"].pop(ch)
                if mt == 1:
                    p01 = pacc.tile([P, ww], BF16, tag=f"cd01_{ch}",
                                    name="p01")
                    nc.gpsimd.tensor_tensor(out=p01[:], in0=prev[:, w],
                                            in1=ex[:, w], op=OP.add)
                    st["p01"][ch] = p01
                else:
                    p23 = pacc.tile([P, ww], BF16, tag=f"cd23_{ch}",
                                    name="p23")
                    nc.gpsimd.tensor_tensor(out=p23[:], in0=prev[:, w],
                                            in1=ex[:, w], op=OP.add)
                    p01 = st["p01"].pop(ch)
                    nc.gpsimd.tensor_tensor(
                        out=csum[:, co:co + ww], in0=p01[:], in1=p23[:],
                        op=OP.add)

        # ---- G unit: (sw, mt) over the full rotated column range ----
        g_state = {"exs": [], "inds": []}

        def g_unit(sw, mt, pg):
            ps = pg.tile([P, SEGW], F32, tag="g", name="g")
            mm(ps, pTn, bTn, mt, sw * SEGW, SEGW)
            ex = pex.tile([P, SEGW], BF16, tag="ex", name="ex")
            nc.scalar.activation(out=ex[:], in_=ps[:], func=AF.Exp,
                                 scale=1.0 / TEMP,
                                 accum_out=sG[:, mt, sw:sw + 1])
            ind = pind.tile([P, SEGW], BF16, tag="ind", name="ind")
            nc.vector.scalar_tensor_tensor(
                out=ind[:], in0=trow[:, sw * SEGW:(sw + 1) * SEGW],
                scalar=invPl[:, mt:mt + 1], in1=ps[:],
                op0=OP.mult, op1=OP.is_lt)
            junk = pjunk.tile([P, SEGW], BF16, tag="junk", name="junk")
            nc.vector.scalar_tensor_tensor(
                out=junk[:], in0=invB[:, sw * SEGW:(sw + 1) * SEGW],
                scalar=ci[:, mt:mt + 1], in1=ps[:],
                op0=OP.mult, op1=OP.is_lt,
                accum_out=cntP[:, mt, sw:sw + 1])
            g_state["exs"].append(ex)
            g_state["inds"].append(ind)
            if mt % 2 == 1:
                k = (mt - 1) // 2
                exs, inds = g_state["exs"], g_state["inds"]
                ea = pacc.tile([P, SEGW], BF16, tag=f"ea{k}", name="ea")
                nc.vector.tensor_tensor(out=ea[:], in0=exs[-2][:],
                                        in1=exs[-1][:], op=OP.add)
                ia = pacc.tile([P, SEGW], BF16, tag=f"ia{k}", name="ia")
                nc.gpsimd.tensor_tensor(out=ia[:], in0=inds[-2][:],
                                        in1=inds[-1][:], op=OP.add)
                base = (sw * 4 + 2 * k) * P
                nc.sync.dma_start(out=gcol_d[base:base + P, :], in_=ia[:])
                nc.sync.dma_start(out=gcol_d[base + P:base + 2 * P, :],
                                  in_=ea[:])
                if mt == 3:
                    g_state["exs"] = []
                    g_state["inds"] = []

        # ---- merged pipeline: 24 C/D units + 16 G units interleaved ----
        cd_units = [(w, mt, ch) for w in ("C", "D") for mt in range(MT)
                    for ch in range(3)]
        g_units = [(sw, mt) for sw in range(NSEGW) for mt in range(MT)]
        with tc.tile_pool(name="pcd", bufs=2, space="PSUM") as pcd, \
             tc.tile_pool(name="pg", bufs=2, space="PSUM") as pg:
            for i in range(8):
                cds = cd_units[3 * i:3 * i + 3]
                gs = g_units[2 * i:2 * i + 2]
                order = [cds[0], gs[0], cds[1], gs[1], cds[2]]
                for u in order:
                    if len(u) == 3:
                        cd_unit(u[0], u[1], u[2], pcd)
                    else:
                        g_unit(u[0], u[1], pg)

        # ---- tail: exports + row-stat reduces ----
        nc.sync.dma_start(out=csum_d[0:P, :], in_=csumC[:])
        nc.sync.dma_start(out=csum_d[P:2 * P, :], in_=csumD[:])
        outA = pstat.tile([P, 4 * MT], F32, tag="outA")
        nc.vector.tensor_reduce(outA[:, 0:MT], cntP[:, :, :],
                                mybir.AxisListType.X, OP.add)
        nc.vector.tensor_reduce(outA[:, MT:2 * MT], sG[:, :, :],
                                mybir.AxisListType.X, OP.add)
        nc.vector.tensor_reduce(outA[:, 2 * MT:3 * MT], sC[:, :, :],
                                mybir.AxisListType.X, OP.add)
        nc.vector.tensor_reduce(outA[:, 3 * MT:4 * MT], sD[:, :, :],
                                mybir.AxisListType.X, OP.add)
        nc.sync.dma_start(out=outA_d[:, :], in_=outA[:])

    nc.compile()
    return nc


@lru_cache(maxsize=1)
def _program():
    return build_program()


def _core_inputs(brand, post):
    nB = np.linalg.norm(brand, axis=1)
    nP = np.linalg.norm(post, axis=1)
    bTn = np.ascontiguousarray((brand / nB[:, None]).T).astype(E4M3)
    pTn = np.ascontiguousarray((post / nP[:, None]).T).astype(E4M3)
    # exact-f32 threshold prep from the quantized data
    diag_full = (bTn.astype(np.float32) * pTn.astype(np.float32)).sum(0)
    trow = (diag_full * nP).astype(np.float32)
    invB = (1.0 / nB).astype(np.float32)
    ident = np.eye(P, dtype=BF16_NP)
    dmask = np.zeros((P, 896), dtype=BF16_NP)
    dmask[np.arange(P), 384 + np.arange(P)] = BNEG
    maps = []
    for c in range(NC):
        rot = c * S
        sl = slice(rot, rot + S)
        maps.append({
            "bTn": np.ascontiguousarray(np.roll(bTn, -rot, axis=1)),
            "pTn": np.ascontiguousarray(np.roll(pTn, -rot, axis=1)[:, :CW]),
            "trow": np.roll(trow, -rot).astype(BF16_NP).reshape(1, N),
            "invB": np.roll(invB, -rot).astype(BF16_NP).reshape(1, N),
            "ident": ident,
            "dmask": dmask,
            "ci": np.ascontiguousarray(
                (diag_full[sl] * nB[sl]).reshape(MT, P).T).astype(np.float32),
            "invPl": np.ascontiguousarray(
                (1.0 / nP[sl]).reshape(MT, P).T).astype(np.float32),
        })
    return maps, diag_full


def kernel(brand, post):
    brand = np.asarray(brand, dtype=np.float32)
    post = np.asarray(post, dtype=np.float32)
    nc = _program()
    maps, diag_full = _core_inputs(brand, post)
    res = run_bass_kernel_spmd(nc, maps, list(range(NC)))

    cnt_post = np.zeros(N)
    sG = np.zeros(N)
    sC = np.zeros(N)
    sD = np.zeros(N)
    colcnt = np.zeros(N)
    colexp = np.zeros(N)
    csumC = np.zeros(N)
    csumD = np.zeros(N)
    for c, r in enumerate(res.results):
        rot = c * S
        sl = slice(rot, rot + S)
        oa = r["outA"].astype(np.float64)

        def unslot(x):
            # slot [p, mt] holds local row mt*128 + p
            return x.T.reshape(S)

        cnt_post[sl] = unslot(oa[:, 0:MT])
        sG[sl] = unslot(oa[:, MT:2 * MT])
        sC[sl] = unslot(oa[:, 2 * MT:3 * MT])
        sD[sl] = unslot(oa[:, 3 * MT:4 * MT])
        gc = r["gcol"].astype(np.float64).reshape(NSEGW, 4, P, SEGW)
        # per sw blocks are [ind01, exp01, ind23, exp23]
        ccnt = (gc[:, 0] + gc[:, 2]).sum(1).reshape(N)
        cexp = (gc[:, 1] + gc[:, 3]).sum(1).reshape(N)
        colcnt += np.roll(ccnt, rot)
        colexp += np.roll(cexp, rot)
        cs = r["csum"].astype(np.float64)
        t = np.zeros(N)
        t[CS0:CS1] = cs[0:P].sum(0)
        csumC += np.roll(t, rot)
        t = np.zeros(N)
        t[CS0:CS1] = cs[P:2 * P].sum(0)
        csumD += np.roll(t, rot)

    d64 = diag_full.astype(np.float64)
    es_post = sG + sD + csumD + 1.0
    es_brand = colexp + sC + csumC + 1.0
    loss_p = (1.0 + 1.0 / (N - cnt_post)) * (np.log(es_post) - d64 / TEMP)
    loss_b = (1.0 + 1.0 / (N - colcnt)) * (np.log(es_brand) - d64 / TEMP)
    return np.float32((loss_b.sum() + loss_p.sum()) / 2.0)


# revision 13
# speedup vs baseline: 1.5703x; 1.0276x over previous
"""CrossCLR intra-modality loss on 8 Trainium2 NeuronCores — v3.

Data-parallel over the 4096-row batch (512 rows/core), fp8-e4m3
normalized embeddings, DoubleRow matmuls. Structure:

* Column rotation: per-core inputs are host-rolled by c*512 columns, so
  each core's slab sits at columns [0, 512) and gram diagonals at the
  fixed position col = mt*128 + p. One SPMD instruction stream.
* Symmetric gram halving: the brand gram C and post gram D are
  symmetric; each core computes only rotated cols [0, 2560) (diag
  block + 4 neighbor slabs; the distance-4 block is computed by both
  partners for their own row sums). Row-sum contributions of the
  uncomputed distance 5..7 blocks come from partners' column sums over
  [512, 2048) (distances 1..3), exchanged through the host.
* Diag masking on PE: a [128, 512] identity-matmul accumulates -16
  onto the diag strip in PSUM, exp underflows to exactly 0 there, and
  the reference's exp(0)=1 diag is restored as +1.0 on the host.
* Merged software pipeline: C, G(=post_l @ brand^T), and D tiles are
  emitted interleaved (PSUM 4+4 banks), so the ACT-heavy gram exp work
  overlaps the DVE-heavy rank-threshold compares of G, keeping every
  engine fed from one end of the kernel to the other.
* No collective and no PE partition-reductions: mt-paired bf16 stat
  tiles (column-count indicators, exp sums) are DMA'd to DRAM and the
  HOST does the partition + cross-core reduction and the O(N)
  rank/log epilogue.

Engines: ACT = all exps (+row-sum accum); DVE = the two rank compares
(+row-count accum) and exp pair-adds; GPSIMD = indicator pair-adds and
gram colsum adds (SBUF-only; GPSIMD cannot touch PSUM); PE = matmuls.
"""

import sys

sys.path.insert(0, "/opt/trn_rl_repo")

from contextlib import ExitStack
from functools import lru_cache

import ml_dtypes
import numpy as np

import concourse.bacc as bacc
import concourse.mybir as mybir
import concourse.tile as tile
from concourse.bass_utils import run_bass_kernel_spmd

N = 4096          # batch rows (global)
D = 1024          # embedding dim
NC = 8            # cores
S = N // NC       # local rows per core (512)
P = 128           # partitions
KC = D // P       # contraction chunks (8)
MT = S // P       # local row tiles (4)
SEG = 512         # PSUM bank width (f32)
SEGW = 1024       # tile width (two banks)
NSEGW = N // SEGW  # 4
CW = 2560         # C/D computed column window (rotated)
CS0, CS1 = 512, 2048  # C/D colsum emission window (distances 1..3)
CWIN = CS1 - CS0  # 1536
TEMP = 0.03
NEG_W = 0.8
BNEG = -16.0      # diag bias: exp(w*(cos-16)/T) underflows to 0

F32 = mybir.dt.float32
BF16 = mybir.dt.bfloat16
FP8 = mybir.dt.float8e4
AF = mybir.ActivationFunctionType
OP = mybir.AluOpType
PM = mybir.MatmulPerfMode.DoubleRow
E4M3 = ml_dtypes.float8_e4m3
BF16_NP = ml_dtypes.bfloat16


def build_program():
    nc = bacc.Bacc("TRN2", target_bir_lowering=False, debug=False, num_devices=NC)

    bTn_d = nc.dram_tensor("bTn", (D, N), FP8, kind="ExternalInput")
    pTn_d = nc.dram_tensor("pTn", (D, CW), FP8, kind="ExternalInput")
    trow_d = nc.dram_tensor("trow", (1, N), BF16, kind="ExternalInput")
    invB_d = nc.dram_tensor("invB", (1, N), BF16, kind="ExternalInput")
    ident_d = nc.dram_tensor("ident", (P, P), BF16, kind="ExternalInput")
    dmask_d = nc.dram_tensor("dmask", (P, 896), BF16, kind="ExternalInput")
    ci_d = nc.dram_tensor("ci", (P, MT), F32, kind="ExternalInput")
    invPl_d = nc.dram_tensor("invPl", (P, MT), F32, kind="ExternalInput")
    outA_d = nc.dram_tensor("outA", (P, 4 * MT), F32, kind="ExternalOutput")
    # per-sw pair tiles: [ind01, ind23, exp01, exp23] x P rows, rotated cols
    gcol_d = nc.dram_tensor("gcol", (NSEGW * 4 * P, SEGW), BF16,
                            kind="ExternalOutput")
    # gram colsums over [CS0, CS1): rows [0:P) = C, [P:2P) = D
    csum_d = nc.dram_tensor("csum", (2 * P, CWIN), BF16, kind="ExternalOutput")

    with tile.TileContext(nc) as tc, ExitStack() as ctx:
        pin = ctx.enter_context(tc.tile_pool(name="pin", bufs=1))
        pstat = ctx.enter_context(tc.tile_pool(name="pstat", bufs=1))
        pex = ctx.enter_context(tc.tile_pool(name="pex", bufs=3))
        pexcd = ctx.enter_context(tc.tile_pool(name="pexcd", bufs=5))
        pind = ctx.enter_context(tc.tile_pool(name="pind", bufs=3))
        pacc = ctx.enter_context(tc.tile_pool(name="pacc", bufs=2))
        pjunk = ctx.enter_context(tc.tile_pool(name="pjunk", bufs=4))

        def load_cols(eng, dst, src_d, c0, c1):
            eng.dma_start(out=dst[:, :, c0:c1],
                          in_=src_d.ap()[:, c0:c1]
                          .rearrange("(k p) n -> p k n", p=P))

        # ---- input loads: one serial DMA resource; priority order matches
        # first use in the merged pipeline ----
        bTn = pin.tile([P, KC, N], FP8, tag="bTn")
        pTn = pin.tile([P, KC, CW], FP8, tag="pTn")
        load_cols(nc.sync, bTn, bTn_d, 0, 256)
        load_cols(nc.sync, bTn, bTn_d, 256, SEG)
        dmask = pin.tile([P, 896], BF16, tag="dmask")
        nc.sync.dma_start(out=dmask[:], in_=dmask_d[:, :])
        ident = pin.tile([P, P], BF16, tag="ident")
        nc.sync.dma_start(out=ident[:], in_=ident_d[:, :])
        load_cols(nc.sync, bTn, bTn_d, SEG, SEGW)
        load_cols(nc.sync, pTn, pTn_d, 0, S)
        trow = pin.tile([P, N], BF16, tag="trow")
        invB = pin.tile([P, N], BF16, tag="invB")
        nc.sync.dma_start(out=trow[:, 0:SEGW],
                          in_=trow_d.ap()[0:1, 0:SEGW].partition_broadcast(P))
        nc.sync.dma_start(out=invB[:, 0:SEGW],
                          in_=invB_d.ap()[0:1, 0:SEGW].partition_broadcast(P))
        ci = pstat.tile([P, MT], F32, tag="ci")
        nc.sync.dma_start(out=ci[:], in_=ci_d[:, :])
        invPl = pstat.tile([P, MT], F32, tag="invPl")
        nc.sync.dma_start(out=invPl[:], in_=invPl_d[:, :])
        load_cols(nc.sync, bTn, bTn_d, SEGW, 2 * SEGW)
        load_cols(nc.sync, bTn, bTn_d, 2 * SEGW, CW)
        nc.sync.dma_start(out=trow[:, SEGW:2 * SEGW],
                          in_=trow_d.ap()[0:1, SEGW:2 * SEGW].partition_broadcast(P))
        nc.sync.dma_start(out=invB[:, SEGW:2 * SEGW],
                          in_=invB_d.ap()[0:1, SEGW:2 * SEGW].partition_broadcast(P))
        load_cols(nc.sync, bTn, bTn_d, CW, 3 * SEGW)
        load_cols(nc.sync, bTn, bTn_d, 3 * SEGW, N)
        nc.sync.dma_start(out=trow[:, 2 * SEGW:N],
                          in_=trow_d.ap()[0:1, 2 * SEGW:N].partition_broadcast(P))
        nc.sync.dma_start(out=invB[:, 2 * SEGW:N],
                          in_=invB_d.ap()[0:1, 2 * SEGW:N].partition_broadcast(P))
        load_cols(nc.sync, pTn, pTn_d, S, 3 * SEG)
        load_cols(nc.sync, pTn, pTn_d, 3 * SEG, 2 * SEGW)
        load_cols(nc.sync, pTn, pTn_d, 2 * SEGW, CW)

        ones = pin.tile([P, 1], BF16, tag="ones")
        nc.vector.memset(ones[:], 1.0)

        cntP, sG = (pstat.tile([P, MT, NSEGW], F32, tag=n, name=n)
                    for n in ("cntP", "sG"))
        sC, sD = (pstat.tile([P, MT, 3], F32, tag=n, name=n)
                  for n in ("sC", "sD"))
        csumC = pstat.tile([P, CWIN], BF16, tag="csumC")
        csumD = pstat.tile([P, CWIN], BF16, tag="csumD")

        def mm(ps, lhsT, rhs, mt, col0, width, open_segs=()):
            for h in range(width // SEG):
                for kk in range(KC // 2):
                    nc.tensor.matmul(
                        ps[:, h * SEG:(h + 1) * SEG],
                        lhsT[:, 2 * kk:2 * kk + 2, mt * P:(mt + 1) * P],
                        rhs[:, 2 * kk:2 * kk + 2,
                            col0 + h * SEG:col0 + (h + 1) * SEG],
                        start=(kk == 0),
                        stop=(kk == KC // 2 - 1 and h not in open_segs),
                        perf_mode=PM)

        # ---- C/D gram unit: (matrix, mt, chunk) with chunks of
        # [0,1024), [1024,2048), [2048,2560) rotated cols ----
        cd_state = {}

        def cd_unit(which, mt, ch, pcd):
            src = bTn if which == "C" else pTn
            slots = sC if which == "C" else sD
            csum = csumC if which == "C" else csumD
            st = cd_state.setdefault(which, {"exs": {}, "p01": {}})
            width = SEG if ch == 2 else SEGW
            ps = pcd.tile([P, SEGW], F32, tag="cd", name="cd")
            mm(ps, src, src, mt, ch * SEGW, width,
               open_segs=(0,) if ch == 0 else ())
            if ch == 0:
                dm = dmask[:, 384 - mt * P:384 - mt * P + SEG]
                nc.tensor.matmul(ps[:, 0:SEG], ident[:], dm,
                                 start=False, stop=True)
            if ch == 2:
                junk = pjunk.tile([P, SEG], BF16, tag="junkcd", name="junkcd")
                nc.scalar.activation(out=junk[:], in_=ps[:, 0:SEG],
                                     func=AF.Exp, scale=NEG_W / TEMP,
                                     accum_out=slots[:, mt, ch:ch + 1])
                return
            ex = pexcd.tile([P, SEGW], BF16, tag="excd", name="excd")
            nc.scalar.activation(out=ex[:], in_=ps[:], func=AF.Exp,
                                 scale=NEG_W / TEMP,
                                 accum_out=slots[:, mt, ch:ch + 1])
            # colsum window: ch0 contributes [512,1024), ch1 all of
            # [1024,2048). Pairwise adds on GPSIMD (SBUF-only engine).
            w = slice(SEG, SEGW) if ch == 0 else slice(0, SEGW)
            ww = SEG if ch == 0 else SEGW
            co = 0 if ch == 0 else SEG  # offset into csum tile
            if mt % 2 == 0:
                st["exs"][ch] = ex
            else:
                prev = st["exs"].pop(ch)
                if mt == 1:
                    p01 = pacc.tile([P, ww], BF16, tag=f"cd01_{ch}",
                                    name="p01")
                    nc.gpsimd.tensor_tensor(out=p01[:], in0=prev[:, w],
                                            in1=ex[:, w], op=OP.add)
                    st["p01"][ch] = p01
                else:
                    p23 = pacc.tile([P, ww], BF16, tag=f"cd23_{ch}",
                                    name="p23")
                    nc.gpsimd.tensor_tensor(out=p23[:], in0=prev[:, w],
                                            in1=ex[:, w], op=OP.add)
                    p01 = st["p01"].pop(ch)
                    nc.gpsimd.tensor_tensor(
                        out=csum[:, co:co + ww], in0=p01[:], in1=p23[:],
                        op=OP.add)

        # ---- G unit: (sw, mt) over the full rotated column range ----
        g_state = {"exs": [], "inds": []}

        def g_unit(sw, mt, pg):
            ps = pg.tile([P, SEGW], F32, tag="g", name="g")
            mm(ps, pTn, bTn, mt, sw * SEGW, SEGW)
            ex = pex.tile([P, SEGW], BF16, tag="ex", name="ex")
            nc.scalar.activation(out=ex[:], in_=ps[:], func=AF.Exp,
                                 scale=1.0 / TEMP,
                                 accum_out=sG[:, mt, sw:sw + 1])
            ind = pind.tile([P, SEGW], BF16, tag="ind", name="ind")
            nc.vector.scalar_tensor_tensor(
                out=ind[:], in0=trow[:, sw * SEGW:(sw + 1) * SEGW],
                scalar=invPl[:, mt:mt + 1], in1=ps[:],
                op0=OP.mult, op1=OP.is_lt)
            junk = pjunk.tile([P, SEGW], BF16, tag="junk", name="junk")
            nc.vector.scalar_tensor_tensor(
                out=junk[:], in0=invB[:, sw * SEGW:(sw + 1) * SEGW],
                scalar=ci[:, mt:mt + 1], in1=ps[:],
                op0=OP.mult, op1=OP.is_lt,
                accum_out=cntP[:, mt, sw:sw + 1])
            g_state["exs"].append(ex)
            g_state["inds"].append(ind)
            if mt % 2 == 1:
                k = (mt - 1) // 2
                exs, inds = g_state["exs"], g_state["inds"]
                ea = pacc.tile([P, SEGW], BF16, tag=f"ea{k}", name="ea")
                nc.vector.tensor_tensor(out=ea[:], in0=exs[-2][:],
                                        in1=exs[-1][:], op=OP.add)
                ia = pacc.tile([P, SEGW], BF16, tag=f"ia{k}", name="ia")
                nc.gpsimd.tensor_tensor(out=ia[:], in0=inds[-2][:],
                                        in1=inds[-1][:], op=OP.add)
                base = (sw * 4 + 2 * k) * P
                nc.sync.dma_start(out=gcol_d[base:base + P, :], in_=ia[:])
                nc.sync.dma_start(out=gcol_d[base + P:base + 2 * P, :],
                                  in_=ea[:])
                if mt == 3:
                    g_state["exs"] = []
                    g_state["inds"] = []

        # ---- merged pipeline: 16 main C/D units (ch0/ch1, the ones that
        # carry colsums) alternate 1:1 with the 16 G units; the 8 dist-4
        # ch2 units (PE+ACT only, no DVE/Pool/export work) run at the end,
        # covering the last G exports and csum drains ----
        cd_main = [(w, mt, ch) for w in ("C", "D") for mt in range(MT)
                   for ch in range(2)]
        cd_tail = [(w, mt, 2) for w in ("C", "D") for mt in range(MT)]
        g_units = [(sw, mt) for sw in range(NSEGW) for mt in range(MT)]
        outA = pstat.tile([P, 4 * MT], F32, tag="outA")
        with tc.tile_pool(name="pcd", bufs=2, space="PSUM") as pcd, \
             tc.tile_pool(name="pg", bufs=2, space="PSUM") as pg:
            for i in range(16):
                cd_unit(*cd_main[i], pcd)
                g_unit(*g_units[i], pg)
                if i == 7:   # C main units done -> csumC final adds queued
                    nc.sync.dma_start(out=csum_d[0:P, :], in_=csumC[:])
            nc.sync.dma_start(out=csum_d[P:2 * P, :], in_=csumD[:])
            # row-count / G-rowsum reduces no longer change after last G
            nc.vector.tensor_reduce(outA[:, 0:MT], cntP[:, :, :],
                                    mybir.AxisListType.X, OP.add)
            nc.vector.tensor_reduce(outA[:, MT:2 * MT], sG[:, :, :],
                                    mybir.AxisListType.X, OP.add)
            for u in cd_tail:
                cd_unit(*u, pcd)

        # ---- tail: remaining row-stat reduces + outA export ----
        nc.vector.tensor_reduce(outA[:, 2 * MT:3 * MT], sC[:, :, :],
                                mybir.AxisListType.X, OP.add)
        nc.vector.tensor_reduce(outA[:, 3 * MT:4 * MT], sD[:, :, :],
                                mybir.AxisListType.X, OP.add)
        nc.sync.dma_start(out=outA_d[:, :], in_=outA[:])

    nc.compile()
    return nc


@lru_cache(maxsize=1)
def _program():
    return build_program()


def _core_inputs(brand, post):
    nB = np.linalg.norm(brand, axis=1)
    nP = np.linalg.norm(post, axis=1)
    bTn = np.ascontiguousarray((brand / nB[:, None]).T).astype(E4M3)
    pTn = np.ascontiguousarray((post / nP[:, None]).T).astype(E4M3)
    # exact-f32 threshold prep from the quantized data
    diag_full = (bTn.astype(np.float32) * pTn.astype(np.float32)).sum(0)
    trow = (diag_full * nP).astype(np.float32)
    invB = (1.0 / nB).astype(np.float32)
    ident = np.eye(P, dtype=BF16_NP)
    dmask = np.zeros((P, 896), dtype=BF16_NP)
    dmask[np.arange(P), 384 + np.arange(P)] = BNEG
    maps = []
    for c in range(NC):
        rot = c * S
        sl = slice(rot, rot + S)
        maps.append({
            "bTn": np.ascontiguousarray(np.roll(bTn, -rot, axis=1)),
            "pTn": np.ascontiguousarray(np.roll(pTn, -rot, axis=1)[:, :CW]),
            "trow": np.roll(trow, -rot).astype(BF16_NP).reshape(1, N),
            "invB": np.roll(invB, -rot).astype(BF16_NP).reshape(1, N),
            "ident": ident,
            "dmask": dmask,
            "ci": np.ascontiguousarray(
                (diag_full[sl] * nB[sl]).reshape(MT, P).T).astype(np.float32),
            "invPl": np.ascontiguousarray(
                (1.0 / nP[sl]).reshape(MT, P).T).astype(np.float32),
        })
    return maps, diag_full


def kernel(brand, post):
    brand = np.asarray(brand, dtype=np.float32)
    post = np.asarray(post, dtype=np.float32)
    nc = _program()
    maps, diag_full = _core_inputs(brand, post)
    res = run_bass_kernel_spmd(nc, maps, list(range(NC)))

    cnt_post = np.zeros(N)
    sG = np.zeros(N)
    sC = np.zeros(N)
    sD = np.zeros(N)
    colcnt = np.zeros(N)
    colexp = np.zeros(N)
    csumC = np.zeros(N)
    csumD = np.zeros(N)
    for c, r in enumerate(res.results):
        rot = c * S
        sl = slice(rot, rot + S)
        oa = r["outA"].astype(np.float64)

        def unslot(x):
            # slot [p, mt] holds local row mt*128 + p
            return x.T.reshape(S)

        cnt_post[sl] = unslot(oa[:, 0:MT])
        sG[sl] = unslot(oa[:, MT:2 * MT])
        sC[sl] = unslot(oa[:, 2 * MT:3 * MT])
        sD[sl] = unslot(oa[:, 3 * MT:4 * MT])
        gc = r["gcol"].astype(np.float64).reshape(NSEGW, 4, P, SEGW)
        # per sw blocks are [ind01, exp01, ind23, exp23]
        ccnt = (gc[:, 0] + gc[:, 2]).sum(1).reshape(N)
        cexp = (gc[:, 1] + gc[:, 3]).sum(1).reshape(N)
        colcnt += np.roll(ccnt, rot)
        colexp += np.roll(cexp, rot)
        cs = r["csum"].astype(np.float64)
        t = np.zeros(N)
        t[CS0:CS1] = cs[0:P].sum(0)
        csumC += np.roll(t, rot)
        t = np.zeros(N)
        t[CS0:CS1] = cs[P:2 * P].sum(0)
        csumD += np.roll(t, rot)

    d64 = diag_full.astype(np.float64)
    es_post = sG + sD + csumD + 1.0
    es_brand = colexp + sC + csumC + 1.0
    loss_p = (1.0 + 1.0 / (N - cnt_post)) * (np.log(es_post) - d64 / TEMP)
    loss_b = (1.0 + 1.0 / (N - colcnt)) * (np.log(es_brand) - d64 / TEMP)
    return np.float32((loss_b.sum() + loss_p.sum()) / 2.0)
